# revision 1
# baseline (speedup 1.0000x reference)
"""DHEL contrastive loss kernel for Trainium2 (8 NeuronCores, SPMD).

Math (reference):
  zhat = z / max(||z||, 1e-12) rowwise;  za = zhat[:8192], zp = zhat[8192:]
  sa_i = sum_j!=i exp(za_i . za_j / tau);  sp_i = sum_j!=i exp(zp_i . zp_j / tau)
  pdot_i = za_i . zp_i
  loss = mean_i( log sa_i + log sp_i - pdot_i / tau )

Sharding: row-parallel over the 8 cores; core c owns anchor rows
[c*1024,(c+1)*1024) and the matching positives. The host hands each core a
row-PERMUTED copy of z ("zn": my anchors, other anchors, my positives, other
positives) plus its contiguous transpose ("zc" = zn.T). The permutation makes
every device-side access pattern core-independent (one NEFF for all cores),
and is harmless because the exp row-sums are invariant to column order within
each half. The host mean over the returned 8x1024 per-row terms is the
unshard step.

Per-core device pipeline (ACT-bound: 2*8192^2/8 = 16.8M exp evaluations at
1 elem/lane/cycle = ~110us minimum on the scalar engine):
  - load zn tiles (natural) and zc column-groups (transposed) as fp32,
    processed in column chunks; the first group is split into 2x1024-wide
    chunks so the exp pipeline ramps early
  - row norms on compact natural tiles (DVE square+reduce, Quake-seed rsqrt
    with two Newton steps -- keeps the ACT stream pure exp, no table reloads)
  - the two ramp chunks normalize in natural layout and PE-TRANSPOSE into the
    matmul layout (PE is idle at startup); later chunks bounce the inverse
    norms through DRAM and return them partition-BROADCAST, making the
    transposed-layout normalize one DVE multiply: zt = bf16(zc * inv_bc)
  - main loop, one column chunk at a time: bf16 matmuls
    (128x128 @ 128x512 -> PSUM fp32) + ONE ACT exp over the whole PSUM chunk
    with accum_out giving the fused row-sum
  - subtract the exact self-similarity term exp(|bf16(zhat_i)|^2/tau)
    (bit-identical to the matmul diagonal), Ln both halves at the end (a
    mid-stream Ln costs two activation-table swaps), add the positive-pair
    dot, DMA out the 1024 per-row loss terms.
"""

import sys

if "/opt/trn_rl_repo" not in sys.path:
    sys.path.insert(0, "/opt/trn_rl_repo")

from contextlib import ExitStack

import numpy as np

import concourse.bass as bass  # noqa: F401  (bass types via bacc)
import concourse.tile as tile
from concourse import bacc, mybir
from concourse.bass_utils import run_bass_kernel_spmd
from concourse.masks import make_identity

P = 128            # SBUF partitions
D = 128            # embedding dim
M = 16384          # total rows of z
HALF = M // 2      # 8192 anchors / positives
NCORES = 8
Q = HALF // NCORES          # 1024 rows per core per half
RC = 8                      # row chunks per half (8 x 128 = 1024)
NSLOT = 5                   # accumulation slots per (half, row-chunk)
TAU = 0.3
SCALE = float(1.0 / TAU)

# column chunks: (global col0, width, half m, accum slot). The first group is
# split in two 1024-wide chunks to shorten the pipeline ramp; chunk 0 also
# holds this core's own anchor rows (the matmul lhsT), chunk 5 the positives.
CHUNKS = [
    (0, 1024, 0, 0),
    (1024, 1024, 0, 4),
    (2048, 2048, 0, 1),
    (4096, 2048, 0, 2),
    (6144, 2048, 0, 3),
    (8192, 2048, 1, 0),
    (10240, 2048, 1, 1),
    (12288, 2048, 1, 2),
    (14336, 2048, 1, 3),
]
LHS_CHUNK = {0: 0, 1: 5}

F32 = mybir.dt.float32
BF16 = mybir.dt.bfloat16
AF = mybir.ActivationFunctionType
OP = mybir.AluOpType
AX = mybir.AxisListType


def _rsqrt_dve(nc, pool, n2, inv, n_tiles):
    """inv = 1/sqrt(n2) on DVE only: Quake seed + 2 Newton steps.

    Seed bits = 0x5f3759df - bits(n2)/2, computed in float arithmetic via
    int<->float value casts (rel err <= ~3.5%); two Newton iterations
    y = y*(1.5 - 0.5*n2*y^2) land at ~5e-6 rel error. Stays entirely off the
    (bottleneck) scalar engine.
    """
    bf = pool.tile([P, n_tiles, 1], F32, tag="nrm_bf")
    nc.vector.tensor_copy(bf[:], n2[:].bitcast(mybir.dt.int32))
    nc.vector.tensor_scalar(
        bf[:], bf[:], -0.5, float(0x5F3759DF), op0=OP.mult, op1=OP.add
    )
    y = pool.tile([P, n_tiles, 1], F32, tag="nrm_y0")
    nc.vector.tensor_copy(y[:].bitcast(mybir.dt.int32), bf[:])
    t0 = pool.tile([P, n_tiles, 1], F32, tag="nrm_t0")
    t1 = pool.tile([P, n_tiles, 1], F32, tag="nrm_t1")
    yn = pool.tile([P, n_tiles, 1], F32, tag="nrm_yn")
    cur = y
    n_it = 2
    for it in range(n_it):
        nc.vector.tensor_tensor(t0[:], cur[:], cur[:], op=OP.mult)
        nc.vector.scalar_tensor_tensor(
            t1[:], t0[:], -0.5, n2[:], op0=OP.mult, op1=OP.mult
        )
        dst = inv if it == n_it - 1 else yn
        nc.vector.scalar_tensor_tensor(
            dst[:], t1[:], 1.5, cur[:], op0=OP.add, op1=OP.mult
        )
        cur = yn


def _build(ctx: ExitStack, tc: tile.TileContext, zn_ext, zc_ext, terms_ext):
    nc = tc.nc

    persist = ctx.enter_context(tc.tile_pool(name="persist", bufs=1))
    zt_pool = ctx.enter_context(tc.tile_pool(name="zt", bufs=1))
    work = ctx.enter_context(tc.tile_pool(name="work", bufs=3))
    invbc_pool = ctx.enter_context(tc.tile_pool(name="invbc", bufs=6))
    eout_pool = ctx.enter_context(tc.tile_pool(name="eout", bufs=2))
    psum_pool = ctx.enter_context(tc.tile_pool(name="psum", bufs=2, space="PSUM"))
    dram_pool = ctx.enter_context(tc.tile_pool(name="dscr", bufs=len(CHUNKS),
                                               space="DRAM"))

    zn_view = zn_ext.rearrange("(t p) d -> p t d", p=P)    # (128, 128, 128)

    # accumulators for the exp row-sum chunks: col = (m*RC + rc)*NSLOT + slot
    accs = persist.tile([P, 2 * RC * NSLOT], F32)
    nc.vector.memset(accs[:], 0.0)   # slot 4 of the positives half stays 0
    # bf16 normalized rows for this core's own rows (matmul-exact replicas)
    zbf_q = [
        persist.tile([P, RC, D], BF16, tag=f"zbfq{m}", name=f"zbfq{m}")
        for m in range(2)
    ]
    selfexp = [
        persist.tile([P, RC], F32, tag=f"selfexp{m}", name=f"selfexp{m}")
        for m in range(2)
    ]
    lnS = [
        persist.tile([P, RC], F32, tag=f"lnS{m}", name=f"lnS{m}")
        for m in range(2)
    ]
    pdot = persist.tile([P, RC, 1], F32)
    ident = persist.tile([P, P], BF16, name="ident")
    make_identity(nc, ident[:])
    zts = [
        zt_pool.tile([P, w], BF16, tag=f"zt{ci}", name=f"zt{ci}")
        for ci, (_, w, _, _) in enumerate(CHUNKS)
    ]

    def half_epilogue(m):
        """Row-sum -> subtract self-term -> Ln for half m (overlappable)."""
        SA = persist.tile([P, RC], F32, tag=f"SA{m}", name=f"SA{m}")
        nc.vector.tensor_reduce(
            SA[:],
            accs[:, m * RC * NSLOT : (m + 1) * RC * NSLOT].rearrange(
                "p (r c) -> p r c", c=NSLOT
            ),
            axis=AX.X, op=OP.add,
        )
        SAadj = persist.tile([P, RC], F32, tag=f"SAadj{m}", name=f"SAadj{m}")
        nc.vector.tensor_tensor(SAadj[:], SA[:], selfexp[m][:], op=OP.subtract)
        nc.scalar.activation(lnS[m][:], SAadj[:], AF.Ln)

    for ci, (col0, W, m, slot) in enumerate(CHUNKS):
        t0i = col0 // P
        ntl = W // P
        # natural tiles for norms (fp32, rotating), transposed column chunk
        znat = work.tile([P, ntl, D], F32, tag="znat")
        nc.sync.dma_start(znat[:], zn_view[:, t0i : t0i + ntl, :])
        if ci >= 2:
            zcol = work.tile([P, W], F32, tag="zcol")
            nc.sync.dma_start(zcol[:], zc_ext[:, col0 : col0 + W])

        # row norms of this chunk's rows
        sqg = work.tile([P, ntl, D], BF16, tag="sqg")
        n2 = work.tile([P, ntl, 1], F32, tag="n2")
        # square+reduce in halves: smaller DVE ops mean the scheduler's
        # greedy idle-fill steals stretch other chunks' serial chains less
        nh = ntl // 2
        for h in range(2):
            nc.vector.tensor_tensor(
                sqg[:, h * nh : (h + 1) * nh, :],
                znat[:, h * nh : (h + 1) * nh, :],
                znat[:, h * nh : (h + 1) * nh, :], op=OP.mult,
            )
            nc.vector.tensor_reduce(
                n2[:, h * nh : (h + 1) * nh, :],
                sqg[:, h * nh : (h + 1) * nh, :], axis=AX.X, op=OP.add,
            )
        inv = work.tile([P, ntl, 1], F32, tag="inv")
        _rsqrt_dve(nc, work, n2, inv, ntl)

        zt = zts[ci][:]
        if ci < 2:
            # ramp chunks: normalize in natural layout and PE-transpose into
            # zt (PE is idle here) -- skips the DRAM broadcast bounce and its
            # SWDGE descriptor serialization on the startup critical path.
            # Values are bit-identical to the bounce path (same f32 inputs,
            # same DVE rounding; the transpose moves bf16 verbatim).
            nat = zbf_q[0] if ci == 0 else work.tile([P, RC, D], BF16,
                                                     tag="nat1", name="nat1")
            nc.vector.tensor_tensor(
                nat[:], znat[:, 0:RC, :],
                inv[:, 0:RC, :].broadcast_to([P, RC, D]), op=OP.mult,
            )
            pst = psum_pool.tile([P, W], BF16, tag="ps", name=f"pst{ci}")
            for t in range(RC):
                nc.tensor.transpose(
                    pst[:, t * P : (t + 1) * P], nat[:, t, :], ident[:]
                )
            nc.vector.tensor_copy(zt, pst[:])
        else:
            # broadcast inv across partitions via a DRAM bounce: write the
            # norms in row order, read them back replicated to all partitions
            invd = dram_pool.tile([W], F32, tag="invd")
            nc.sync.dma_start(invd[:].rearrange("(t p) -> p t", p=P), inv[:])
            invbc = invbc_pool.tile([P, W], F32, tag="invbc")
            nc.gpsimd.dma_start(invbc[:], invd[:].partition_broadcast(P))
            # normalized bf16 column chunk (cast fused into the multiply)
            nc.vector.tensor_tensor(zt, zcol[:], invbc[:], op=OP.mult)

        # ---- main loop block for this column chunk ----
        lhsrc = zts[LHS_CHUNK[m]][:]
        for rc in range(RC):
            ps = psum_pool.tile([P, W], F32, tag="ps")
            for k in range(W // 512):
                nc.tensor.matmul(
                    ps[:, k * 512 : (k + 1) * 512],
                    lhsrc[:, rc * P : (rc + 1) * P],
                    zt[:, k * 512 : (k + 1) * 512],
                    start=True,
                    stop=True,
                )
            eo = eout_pool.tile([P, W], F32, tag="eo")
            col = (m * RC + rc) * NSLOT + slot
            nc.scalar.activation(
                eo[:], ps[:], AF.Exp, scale=SCALE,
                accum_out=accs[:, col : col + 1],
            )

        if ci in (0, 5):
            # my rows are tiles 0..7 of chunk 0 (anchors) / chunk 5
            # (positives): matmul-exact bf16 replica for pdot/self-sim
            # (chunk 0's was already built for the transpose path above)
            if ci == 5:
                nc.vector.tensor_tensor(
                    zbf_q[m][:],
                    znat[:, 0:RC, :],
                    inv[:, 0:RC, :].broadcast_to([P, RC, D]),
                    op=OP.mult,
                )
            sq2 = persist.tile([P, RC, D], F32, tag=f"sq2_{m}",
                               name=f"sq2_{m}")
            nc.vector.tensor_tensor(sq2[:], zbf_q[m][:], zbf_q[m][:],
                                    op=OP.mult)
            selfsq = persist.tile([P, RC, 1], F32, tag=f"selfsq{m}",
                                  name=f"selfsq{m}")
            nc.vector.tensor_reduce(selfsq[:], sq2[:], axis=AX.X, op=OP.add)
            nc.scalar.activation(
                selfexp[m][:].rearrange("p (r o) -> p r o", o=1),
                selfsq[:], AF.Exp, scale=SCALE,
            )
        if ci == 5:
            # pdot straight from the bf16 normalized vectors
            prodq = persist.tile([P, RC, D], F32, tag="prodq")
            nc.vector.tensor_tensor(prodq[:], zbf_q[0][:], zbf_q[1][:],
                                    op=OP.mult)
            nc.vector.tensor_reduce(pdot[:], prodq[:], axis=AX.X, op=OP.add)
    # both halves' logs at the end: a mid-stream Ln would force two
    # activation-table swaps (~2.6us) inside the exp stream
    half_epilogue(0)
    half_epilogue(1)

    # ---------------- final combine ----------------
    tsum = persist.tile([P, RC], F32)
    nc.vector.tensor_tensor(tsum[:], lnS[0][:], lnS[1][:], op=OP.add)
    terms = persist.tile([P, RC], F32)
    # terms = (pdot * -1/tau) + (ln sa + ln sp)
    nc.vector.scalar_tensor_tensor(
        terms[:], pdot[:].rearrange("p t o -> p (t o)"), -SCALE, tsum[:],
        op0=OP.mult, op1=OP.add,
    )
    nc.sync.dma_start(terms_ext.rearrange("t p -> p t"), terms[:])


def build_kernel() -> bass.Bass:
    nc = bacc.Bacc("TRN2", target_bir_lowering=False, debug=False,
                   num_devices=NCORES)
    zn_ext = nc.dram_tensor("zn", (M, D), F32, kind="ExternalInput").ap()
    zc_ext = nc.dram_tensor("zc", (D, M), F32, kind="ExternalInput").ap()
    terms_ext = nc.dram_tensor("terms", (RC, P), F32, kind="ExternalOutput").ap()
    with tile.TileContext(nc) as tc:
        with ExitStack() as ctx:
            _build(ctx, tc, zn_ext, zc_ext, terms_ext)
    nc.compile()
    return nc


_CACHE: dict = {}


def kernel(z, _trace: bool = False):
    z = np.ascontiguousarray(np.asarray(z, dtype=np.float32))
    assert z.shape == (M, D), z.shape
    if "nc" not in _CACHE:
        _CACHE["nc"] = build_kernel()
    nc = _CACHE["nc"]

    za, zp = z[:HALF], z[HALF:]
    in_maps = []
    for c in range(NCORES):
        sel = np.r_[c * Q : (c + 1) * Q, 0 : c * Q, (c + 1) * Q : HALF]
        zn = np.concatenate([za[sel], zp[sel]], axis=0)
        zc = np.ascontiguousarray(zn.T)
        in_maps.append({"zn": np.ascontiguousarray(zn), "zc": zc})

    res = run_bass_kernel_spmd(
        nc, in_maps, core_ids=list(range(NCORES)), trace=_trace
    )
    _CACHE["last_results"] = res
    terms = np.concatenate(
        [r["terms"].astype(np.float64).reshape(-1) for r in res.results]
    )
    return np.float32(terms.mean())



# revision 23
# speedup vs baseline: 1.8772x; 1.8772x over previous
"""DHEL contrastive loss kernel for Trainium2 (8 NeuronCores, SPMD).

Math (reference):
  zhat = z / max(||z||, 1e-12) rowwise;  za = zhat[:8192], zp = zhat[8192:]
  sa_i = sum_j!=i exp(za_i . za_j / tau);  sp_i = sum_j!=i exp(zp_i . zp_j / tau)
  pdot_i = za_i . zp_i
  loss = mean_i( log sa_i + log sp_i - pdot_i / tau )

Key idea vs the naive row-parallel split: the two similarity matrices are
SYMMETRIC, so only the upper triangle of 1024x1024 blocks needs the (scalar-
engine-bound) exp evaluation. Each computed off-diagonal block (R, C)
contributes its row-sums to rows R (free via the activation accumulator) and
its column-sums to rows C (bf16 tree-add on DVE + a partition-reduce on the
otherwise idle Pool engine). Per-core exp work drops from 2*8192^2/8 = 16.8M
elements to 9 blocks = 9.4M, and the scalar engine runs wide (2048-col)
back-to-back exp+accumulate over the whole kernel.

Work assignment: a tournament orientation of K8 gives every core exactly 9
blocks: its own diagonal block in each half, its in-star pairs in one half and
out-star pairs in the other (4 + 3). Cores 0-3 get [5 anchor blocks, 4
positive], cores 4-7 the mirror image; the device program is identical
("X half" = 5 column blocks, "Y half" = 4) and the host maps halves/blocks
per core and inverts the mapping when assembling.

Division of labor (per the sharding hint, devices consume all-gathered
NORMALIZED embeddings): the host normalizes rows in f64, casts to bf16, and
ships each core the transposed [d, columns] slice it contracts against
(2.3 MB/core); block-granular DMAs so the first matmul issues ~4 us in. The
O(N^2 d) similarity/exp/reduction work all happens on device. The host folds
the returned row/column partial sums, subtracts the (exactly reproducible)
bf16 self-similarity, adds the positive-pair dots, and takes log+mean in f64
-- O(N d) assembly, 0.03% of the FLOPs.
"""

import sys

if "/opt/trn_rl_repo" not in sys.path:
    sys.path.insert(0, "/opt/trn_rl_repo")

from contextlib import ExitStack

import numpy as np

import concourse.bass as bass  # noqa: F401
import concourse.tile as tile
from concourse import bacc, mybir
from concourse.bass_utils import run_bass_kernel_spmd

P = 128
D = 128
M = 16384
HALF = M // 2       # 8192
Q = 1024            # rows per block
NCORES = 8
NT = 72             # 9 blocks x 8 tiles of 128 rows
TAU = 0.3
SCALE = float(1.0 / TAU)

# column groups: (lhs base col, rhs col offset, width, [colacc ids], tri)
# Local zt column blocks: [Cx, x1..x4, Cy, y1..y3] at 1024 cols each.
# colacc ids 0..3 are X partners x1..x4, 4..6 are Y partners y1..y3; diag
# slices have no block colacc. The Cy+y1 group runs LAST so the tail is
# short. tri=True groups start at the diagonal block: row-chunk rc only
# computes columns >= rc*128 (the block is symmetric, the lower triangle is
# recovered as column sums -> colout slots 8 (Cx) / 9 (Cy) cover block
# columns 128..1024 from the chunks above them).
GROUPS = [
    (0, 0, 1024, [], True),             # Cx (diag)
    (0, 1024, 2048, [0, 1], False),     # x1 x2
    (0, 3072, 2048, [2, 3], False),     # x3 x4
    (5120, 7168, 2048, [5, 6], False),  # y2 y3
    (5120, 5120, 2048, [-1, 4], True),  # Cy (diag) + y1
]
# DMA arrival order for the 9 column blocks (group consumption order).
DMA_ORDER = (0, 1, 2, 3, 4, 5, 7, 8, 6)

F32 = mybir.dt.float32
BF16 = mybir.dt.bfloat16
AF = mybir.ActivationFunctionType
OP = mybir.AluOpType
AX = mybir.AxisListType


def _pair_owner(i: int, j: int) -> int:
    """Tournament owner of pair {i,j}, i<j: in-degrees (4,4,4,4,3,3,3,3)."""
    if j < 7:
        return i if (j - i) % 7 in (1, 2, 3) else j
    return i if i <= 3 else 7


def _core_layout(c: int):
    """Returns (x_is_anchor, xblocks, yblocks): block ids of the X (5-block)
    and Y (4-block) halves, center first."""
    a_star = []   # pairs {c,x} owned by c -> anchor-half partners
    p_star = []   # pairs not owned by c  -> positive-half partners
    for x in range(8):
        if x == c:
            continue
        i, j = min(c, x), max(c, x)
        (a_star if _pair_owner(i, j) == c else p_star).append(x)
    if len(a_star) == 4:
        return True, [c] + a_star, [c] + p_star
    return False, [c] + p_star, [c] + a_star


def _build(ctx: ExitStack, tc: tile.TileContext, zc_ext, rows_ext, cols_ext):
    nc = tc.nc

    persist = ctx.enter_context(tc.tile_pool(name="persist", bufs=1))
    eo_pool = ctx.enter_context(tc.tile_pool(name="eo", bufs=4))
    psum_pool = ctx.enter_context(tc.tile_pool(name="psum", bufs=2,
                                               space="PSUM"))

    zt = persist.tile([P, NT * P], BF16, name="zt")
    rows = persist.tile([P, 5, 8], F32, name="rows")
    colacc = [persist.tile([P, Q], BF16, name=f"colacc{b}") for b in range(7)]
    colaccD = [persist.tile([P, 896], BF16, name=f"colaccD{m}")
               for m in range(2)]
    colout = persist.tile([1, 10, Q], F32, name="colout")
    for k in DMA_ORDER:
        nc.sync.dma_start(zt[:, k * Q : (k + 1) * Q],
                          zc_ext[:, k * Q : (k + 1) * Q])

    def colsum_out(slot, src, width=Q):
        nc.gpsimd.tensor_reduce(colout[:, slot, 0:width], src, axis=AX.C,
                                op=OP.add)
        nc.sync.dma_start(
            cols_ext[slot * Q : slot * Q + width].rearrange("(o j) -> o j",
                                                            o=1),
            colout[:, slot, 0:width],
        )

    def sweep(g):
        lhs0, off, w, caccs, tri = GROUPS[g]
        last = g == len(GROUPS) - 1
        dacc = colaccD[0 if g == 0 else 1]
        for rc in range(8):
            cut = 128 * rc if tri else 0
            wr = w - cut
            ps = psum_pool.tile([P, wr], F32, tag="ps", name=f"ps_g{g}_{rc}")
            col = 0
            while col < wr:
                cw = min(512, wr - col)
                nc.tensor.matmul(
                    ps[:, col : col + cw],
                    zt[:, lhs0 + rc * P : lhs0 + (rc + 1) * P],
                    zt[:, off + cut + col : off + cut + col + cw],
                    start=True, stop=True,
                )
                col += cw
            eo = eo_pool.tile([P, wr], BF16, tag="eo", name=f"eo_g{g}_{rc}")
            nc.scalar.activation(
                eo[:], ps[:], AF.Exp, scale=SCALE,
                accum_out=rows[:, g, rc : rc + 1],
            )
            if tri and rc < 7:
                # below-diagonal coverage of the diag block: columns
                # [(rc+1)*128, 1024) accumulate as column sums
                sl = eo[:, 128 : Q - cut]
                dst = dacc[:, rc * 128 : 896]
                if rc == 0:
                    nc.vector.tensor_copy(dst, sl)
                else:
                    nc.vector.tensor_tensor(dst, dst, sl, op=OP.add)
            for i, b in enumerate(caccs):
                if b < 0:
                    continue
                sl = eo[:, i * Q - cut : (i + 1) * Q - cut]
                if last and rc == 7:
                    # tail: partition-reduce the final chunk directly from eo
                    # (no tree-add), shortening the post-activation chain.
                    colsum_out(7, sl)
                elif rc == 0:
                    nc.vector.tensor_copy(colacc[b][:], sl)
                else:
                    nc.vector.tensor_tensor(colacc[b][:], colacc[b][:], sl,
                                            op=OP.add)
            if last and rc == 6:
                # pre-reduce the tail block's first 7 chunks while rc 7 runs,
                # and the diag column sums (complete after rc 6's add)
                colsum_out(caccs[1], colacc[caccs[1]][:])
                colsum_out(9, dacc[:], 896)
        if not last:
            if tri:
                colsum_out(8, dacc[:], 896)
            for b in caccs:
                if b >= 0:
                    colsum_out(b, colacc[b][:])

    for g in range(len(GROUPS)):
        sweep(g)

    nc.sync.dma_start(rows_ext.rearrange("p (g r) -> p g r", g=5), rows[:])


def build_kernel() -> bass.Bass:
    nc = bacc.Bacc("TRN2", target_bir_lowering=False, debug=False,
                   num_devices=NCORES)
    zc_ext = nc.dram_tensor("zc", (D, NT * P), BF16, kind="ExternalInput").ap()
    rows_ext = nc.dram_tensor("rows", (P, 40), F32, kind="ExternalOutput").ap()
    # cols slots 0..6: full colacc column sums; slot 7: the tail block's rc7
    # chunk (added into y1's total); slots 8/9: the Cx/Cy diag blocks'
    # below-diagonal column sums (block columns 128..1024, width 896).
    cols_ext = nc.dram_tensor("cols", (10 * Q,), F32,
                              kind="ExternalOutput").ap()
    with tile.TileContext(nc) as tc:
        with ExitStack() as ctx:
            _build(ctx, tc, zc_ext, rows_ext, cols_ext)
    nc.compile()
    return nc


def _normalized_bf16(z: np.ndarray) -> np.ndarray:
    import ml_dtypes

    zf = np.asarray(z, dtype=np.float64)
    zf = zf / np.maximum(np.linalg.norm(zf, axis=1, keepdims=True), 1e-12)
    return zf.astype(ml_dtypes.bfloat16)


def make_in_map(zhat_bf: np.ndarray, c: int) -> dict:
    """Build core c's transposed bf16 input: X blocks then Y blocks."""
    za, zp = zhat_bf[:HALF], zhat_bf[HALF:]
    x_is_anchor, xblocks, yblocks = _core_layout(c)
    xsrc, ysrc = (za, zp) if x_is_anchor else (zp, za)
    parts = [xsrc[b * Q : (b + 1) * Q] for b in xblocks]
    parts += [ysrc[b * Q : (b + 1) * Q] for b in yblocks]
    zc = np.ascontiguousarray(np.concatenate(parts, axis=0).T)
    return {"zc": zc}


def assemble(zhat_bf: np.ndarray, outs: list) -> np.float32:
    """Host-side O(N d) assembly of the per-core partials into the loss."""
    zf = zhat_bf.astype(np.float64)
    za, zp = zf[:HALF], zf[HALF:]
    Sa = np.zeros(HALF, dtype=np.float64)
    Sp = np.zeros(HALF, dtype=np.float64)
    for c in range(NCORES):
        o = outs[c]
        rows = np.asarray(o["rows"], dtype=np.float64)    # (128, 40)
        cols = np.asarray(o["cols"], dtype=np.float64)    # (10240,)
        x_is_anchor, xblocks, yblocks = _core_layout(c)
        SX, SY = (Sa, Sp) if x_is_anchor else (Sp, Sa)
        # rows[p, g*8+rc] belongs to center-block row rc*128+p
        r = rows.reshape(P, 5, 8).transpose(2, 0, 1).reshape(Q, 5)
        base = c * Q
        SX[base : base + Q] += r[:, 0] + r[:, 1] + r[:, 2]
        SY[base : base + Q] += r[:, 3] + r[:, 4]
        cols = cols.reshape(10, Q)
        for i, b in enumerate(xblocks[1:]):
            SX[b * Q : (b + 1) * Q] += cols[i]
        # y1 gets the pre-reduced rc0-6 partial plus the separate rc7 chunk
        y1, y2, y3 = yblocks[1], yblocks[2], yblocks[3]
        SY[y2 * Q : (y2 + 1) * Q] += cols[5]
        SY[y3 * Q : (y3 + 1) * Q] += cols[6]
        SY[y1 * Q : (y1 + 1) * Q] += cols[4] + cols[7]
        # diag blocks' below-diagonal coverage (columns 128..1024)
        SX[base + 128 : base + Q] += cols[8][0:896]
        SY[base + 128 : base + Q] += cols[9][0:896]
    # self-similarity: the diagonal the device summed is sum_d bf16(zhat)^2
    # accumulated in f32 -- reproduce it (up to f32 summation order) here
    selfa = np.exp(np.sum(za * za, axis=1) * SCALE)
    selfp = np.exp(np.sum(zp * zp, axis=1) * SCALE)
    pdot = np.sum(za * zp, axis=1)
    terms = (np.log(Sa - selfa) + np.log(Sp - selfp) - pdot * SCALE)
    return np.float32(terms.mean())


_CACHE: dict = {}


def kernel(z, _trace: bool = False):
    z = np.ascontiguousarray(np.asarray(z, dtype=np.float32))
    assert z.shape == (M, D), z.shape
    if "nc" not in _CACHE:
        _CACHE["nc"] = build_kernel()
    nc = _CACHE["nc"]

    zhat_bf = _normalized_bf16(z)
    in_maps = [make_in_map(zhat_bf, c) for c in range(NCORES)]
    res = run_bass_kernel_spmd(
        nc, in_maps, core_ids=list(range(NCORES)), trace=_trace
    )
    _CACHE["last_results"] = res
    return assemble(zhat_bf, res.results)


# revision 29
# speedup vs baseline: 2.0308x; 1.0818x over previous
"""DHEL contrastive loss kernel for Trainium2 (8 NeuronCores, SPMD).

Math (reference):
  zhat = z / max(||z||, 1e-12) rowwise;  za = zhat[:8192], zp = zhat[8192:]
  sa_i = sum_j!=i exp(za_i . za_j / tau);  sp_i = sum_j!=i exp(zp_i . zp_j / tau)
  pdot_i = za_i . zp_i
  loss = mean_i( log sa_i + log sp_i - pdot_i / tau )

Key idea vs the naive row-parallel split: the two similarity matrices are
SYMMETRIC, so only the upper triangle of 1024x1024 blocks needs the (scalar-
engine-bound) exp evaluation. Each computed off-diagonal block (R, C)
contributes its row-sums to rows R (free via the activation accumulator) and
its column-sums to rows C (bf16 tree-add on DVE + a partition-reduce on the
otherwise idle Pool engine). Per-core exp work drops from 2*8192^2/8 = 16.8M
elements to 9 blocks = 9.4M, and the scalar engine runs wide (2048-col)
back-to-back exp+accumulate over the whole kernel.

Work assignment: a tournament orientation of K8 gives every core exactly 9
blocks: its own diagonal block in each half, its in-star pairs in one half and
out-star pairs in the other (4 + 3). Cores 0-3 get [5 anchor blocks, 4
positive], cores 4-7 the mirror image; the device program is identical
("X half" = 5 column blocks, "Y half" = 4) and the host maps halves/blocks
per core and inverts the mapping when assembling.

Division of labor (per the sharding hint, devices consume all-gathered
NORMALIZED embeddings): the host normalizes rows in f64, casts to bf16, and
ships each core the transposed [d, columns] slice it contracts against
(2.3 MB/core); block-granular DMAs so the first matmul issues ~4 us in. The
O(N^2 d) similarity/exp/reduction work all happens on device. The host folds
the returned row/column partial sums, subtracts the (exactly reproducible)
bf16 self-similarity, adds the positive-pair dots, and takes log+mean in f64
-- O(N d) assembly, 0.03% of the FLOPs.
"""

import sys

if "/opt/trn_rl_repo" not in sys.path:
    sys.path.insert(0, "/opt/trn_rl_repo")

from contextlib import ExitStack

import numpy as np

import concourse.bass as bass  # noqa: F401
import concourse.tile as tile
from concourse import bacc, mybir
from concourse.bass_utils import run_bass_kernel_spmd

P = 128
D = 128
M = 16384
HALF = M // 2       # 8192
Q = 1024            # rows per block
NCORES = 8
NT = 72             # 9 blocks x 8 tiles of 128 rows
TAU = 0.3
SCALE = float(1.0 / TAU)

# column groups: (lhs base col, rhs col offset, width, [colacc ids], tri)
# Local zt column blocks: [Cx, x1..x4, Cy, y1..y3] at 1024 cols each.
# colacc ids 0..3 are X partners x1..x4, 4..6 are Y partners y1..y3; diag
# slices have no block colacc. The Cy+y1 group runs LAST so the tail is
# short. tri=True groups start at the diagonal block: row-chunk rc only
# computes columns >= rc*128 (the block is symmetric, the lower triangle is
# recovered as column sums -> colout slots 8 (Cx) / 9 (Cy) cover block
# columns 128..1024 from the chunks above them).
GROUPS = [
    (0, 0, 1024, [], True),             # Cx (diag)
    (0, 1024, 2048, [0, 1], False),     # x1 x2
    (0, 3072, 2048, [2, 3], False),     # x3 x4
    (5120, 7168, 2048, [5, 6], False),  # y2 y3
    (5120, 5120, 2048, [-1, 4], True),  # Cy (diag) + y1
]
# DMA arrival order for the 9 column blocks (group consumption order).
DMA_ORDER = (0, 1, 2, 3, 4, 5, 7, 8, 6)

F32 = mybir.dt.float32
BF16 = mybir.dt.bfloat16
AF = mybir.ActivationFunctionType
OP = mybir.AluOpType
AX = mybir.AxisListType


def _pair_owner(i: int, j: int) -> int:
    """Tournament owner of pair {i,j}, i<j: in-degrees (4,4,4,4,3,3,3,3)."""
    if j < 7:
        return i if (j - i) % 7 in (1, 2, 3) else j
    return i if i <= 3 else 7


def _core_layout(c: int):
    """Returns (x_is_anchor, xblocks, yblocks): block ids of the X (5-block)
    and Y (4-block) halves, center first."""
    a_star = []   # pairs {c,x} owned by c -> anchor-half partners
    p_star = []   # pairs not owned by c  -> positive-half partners
    for x in range(8):
        if x == c:
            continue
        i, j = min(c, x), max(c, x)
        (a_star if _pair_owner(i, j) == c else p_star).append(x)
    if len(a_star) == 4:
        return True, [c] + a_star, [c] + p_star
    return False, [c] + p_star, [c] + a_star


def _build(ctx: ExitStack, tc: tile.TileContext, zc_ext, rows_ext, cols_ext,
           etail_ext):
    nc = tc.nc

    persist = ctx.enter_context(tc.tile_pool(name="persist", bufs=1))
    eo_pool = ctx.enter_context(tc.tile_pool(name="eo", bufs=4))
    psum_pool = ctx.enter_context(tc.tile_pool(name="psum", bufs=2,
                                               space="PSUM"))

    zt = persist.tile([P, NT * P], BF16, name="zt")
    rows = persist.tile([P, 5, 8], F32, name="rows")
    colacc = [persist.tile([P, Q], BF16, name=f"colacc{b}") for b in range(7)]
    colaccD = [persist.tile([P, 896], BF16, name=f"colaccD{m}")
               for m in range(2)]
    colout = persist.tile([1, 10, Q], F32, name="colout")
    for k in DMA_ORDER:
        nc.sync.dma_start(zt[:, k * Q : (k + 1) * Q],
                          zc_ext[:, k * Q : (k + 1) * Q])

    def colsum_out(slot, src, off=0, width=Q):
        nc.gpsimd.tensor_reduce(colout[:, slot, off : off + width], src,
                                axis=AX.C, op=OP.add)
        nc.sync.dma_start(
            cols_ext[slot * Q + off : slot * Q + off + width].rearrange(
                "(o j) -> o j", o=1),
            colout[:, slot, off : off + width],
        )

    def sweep(g):
        lhs0, off, w, caccs, tri = GROUPS[g]
        last = g == len(GROUPS) - 1
        dacc = colaccD[0 if g == 0 else 1]
        for rc in range(8):
            cut = 128 * rc if tri else 0
            wr = w - cut
            ps = psum_pool.tile([P, wr], F32, tag="ps", name=f"ps_g{g}_{rc}")
            col = 0
            while col < wr:
                cw = min(512, wr - col)
                nc.tensor.matmul(
                    ps[:, col : col + cw],
                    zt[:, lhs0 + rc * P : lhs0 + (rc + 1) * P],
                    zt[:, off + cut + col : off + cut + col + cw],
                    start=True, stop=True,
                )
                col += cw
            eo = eo_pool.tile([P, wr], BF16, tag="eo", name=f"eo_g{g}_{rc}")
            nc.scalar.activation(
                eo[:], ps[:], AF.Exp, scale=SCALE,
                accum_out=rows[:, g, rc : rc + 1],
            )
            if tri and rc < 7:
                # below-diagonal coverage of the diag block: columns
                # [(rc+1)*128, 1024) accumulate as column sums
                sl = eo[:, 128 : Q - cut]
                dst = dacc[:, rc * 128 : 896]
                if rc == 0:
                    nc.vector.tensor_copy(dst, sl)
                else:
                    nc.vector.tensor_tensor(dst, dst, sl, op=OP.add)
            for i, b in enumerate(caccs):
                if b < 0:
                    continue
                sl = eo[:, i * Q - cut : (i + 1) * Q - cut]
                if last and rc >= 6:
                    # tail: ship the raw bf16 chunk to DRAM; the host sums
                    # the 128 partitions -- no post-activation engine work.
                    a = rc - 6
                    nc.scalar.dma_start(etail_ext[a * P : (a + 1) * P, :], sl)
                elif rc == 0:
                    nc.vector.tensor_copy(colacc[b][:], sl)
                else:
                    nc.vector.tensor_tensor(colacc[b][:], colacc[b][:], sl,
                                            op=OP.add)
            if last and rc == 5:
                # pre-reduce everything the tail depends on while chunks 6/7
                # still run: y1's rc0-5 partial and the diag columns below
                # 768 (rc6's add only touches 768..896)
                colsum_out(caccs[1], colacc[caccs[1]][:])
                colsum_out(9, dacc[:, 0:768], 0, 768)
            if last and rc == 6:
                colsum_out(9, dacc[:, 768:896], 768, 128)
        if not last:
            if tri:
                colsum_out(8, dacc[:], 0, 896)
            for b in caccs:
                if b >= 0:
                    colsum_out(b, colacc[b][:])

    for g in range(len(GROUPS)):
        sweep(g)

    nc.scalar.dma_start(rows_ext.rearrange("p (g r) -> p g r", g=5),
                        rows[:])


def build_kernel() -> bass.Bass:
    nc = bacc.Bacc("TRN2", target_bir_lowering=False, debug=False,
                   num_devices=NCORES)
    zc_ext = nc.dram_tensor("zc", (D, NT * P), BF16, kind="ExternalInput").ap()
    rows_ext = nc.dram_tensor("rows", (P, 40), F32, kind="ExternalOutput").ap()
    # cols slots 0..6: full colacc column sums; slot 7: the tail block's rc7
    # chunk (added into y1's total); slots 8/9: the Cx/Cy diag blocks'
    # below-diagonal column sums (block columns 128..1024, width 896).
    cols_ext = nc.dram_tensor("cols", (10 * Q,), F32,
                              kind="ExternalOutput").ap()
    # raw bf16 eo chunks 6/7 of the tail block's y1 slice (host sums them)
    etail_ext = nc.dram_tensor("etail", (2 * P, Q), BF16,
                               kind="ExternalOutput").ap()
    with tile.TileContext(nc) as tc:
        with ExitStack() as ctx:
            _build(ctx, tc, zc_ext, rows_ext, cols_ext, etail_ext)
    nc.compile()
    return nc


def _normalized_bf16(z: np.ndarray) -> np.ndarray:
    import ml_dtypes

    zf = np.asarray(z, dtype=np.float64)
    zf = zf / np.maximum(np.linalg.norm(zf, axis=1, keepdims=True), 1e-12)
    return zf.astype(ml_dtypes.bfloat16)


def make_in_map(zhat_bf: np.ndarray, c: int) -> dict:
    """Build core c's transposed bf16 input: X blocks then Y blocks."""
    za, zp = zhat_bf[:HALF], zhat_bf[HALF:]
    x_is_anchor, xblocks, yblocks = _core_layout(c)
    xsrc, ysrc = (za, zp) if x_is_anchor else (zp, za)
    parts = [xsrc[b * Q : (b + 1) * Q] for b in xblocks]
    parts += [ysrc[b * Q : (b + 1) * Q] for b in yblocks]
    zc = np.ascontiguousarray(np.concatenate(parts, axis=0).T)
    return {"zc": zc}


def assemble(zhat_bf: np.ndarray, outs: list) -> np.float32:
    """Host-side O(N d) assembly of the per-core partials into the loss."""
    zf = zhat_bf.astype(np.float64)
    za, zp = zf[:HALF], zf[HALF:]
    Sa = np.zeros(HALF, dtype=np.float64)
    Sp = np.zeros(HALF, dtype=np.float64)
    for c in range(NCORES):
        o = outs[c]
        rows = np.asarray(o["rows"], dtype=np.float64)    # (128, 40)
        cols = np.asarray(o["cols"], dtype=np.float64)    # (10240,)
        x_is_anchor, xblocks, yblocks = _core_layout(c)
        SX, SY = (Sa, Sp) if x_is_anchor else (Sp, Sa)
        # rows[p, g*8+rc] belongs to center-block row rc*128+p
        r = rows.reshape(P, 5, 8).transpose(2, 0, 1).reshape(Q, 5)
        base = c * Q
        SX[base : base + Q] += r[:, 0] + r[:, 1] + r[:, 2]
        SY[base : base + Q] += r[:, 3] + r[:, 4]
        cols = cols.reshape(10, Q)
        for i, b in enumerate(xblocks[1:]):
            SX[b * Q : (b + 1) * Q] += cols[i]
        # y1 gets the pre-reduced rc0-6 partial plus the separate rc7 chunk
        y1, y2, y3 = yblocks[1], yblocks[2], yblocks[3]
        SY[y2 * Q : (y2 + 1) * Q] += cols[5]
        SY[y3 * Q : (y3 + 1) * Q] += cols[6]
        etail = np.asarray(o["etail"], dtype=np.float64).reshape(2, P, Q)
        SY[y1 * Q : (y1 + 1) * Q] += (cols[4] + etail[0].sum(axis=0)
                                      + etail[1].sum(axis=0))
        # diag blocks' below-diagonal coverage (columns 128..1024)
        SX[base + 128 : base + Q] += cols[8][0:896]
        SY[base + 128 : base + Q] += cols[9][0:896]
    # self-similarity: the diagonal the device summed is sum_d bf16(zhat)^2
    # accumulated in f32 -- reproduce it (up to f32 summation order) here
    selfa = np.exp(np.sum(za * za, axis=1) * SCALE)
    selfp = np.exp(np.sum(zp * zp, axis=1) * SCALE)
    pdot = np.sum(za * zp, axis=1)
    terms = (np.log(Sa - selfa) + np.log(Sp - selfp) - pdot * SCALE)
    return np.float32(terms.mean())


_CACHE: dict = {}


def kernel(z, _trace: bool = False):
    z = np.ascontiguousarray(np.asarray(z, dtype=np.float32))
    assert z.shape == (M, D), z.shape
    if "nc" not in _CACHE:
        _CACHE["nc"] = build_kernel()
    nc = _CACHE["nc"]

    zhat_bf = _normalized_bf16(z)
    in_maps = [make_in_map(zhat_bf, c) for c in range(NCORES)]
    res = run_bass_kernel_spmd(
        nc, in_maps, core_ids=list(range(NCORES)), trace=_trace
    )
    _CACHE["last_results"] = res
    return assemble(zhat_bf, res.results)
